# revision 16
# baseline (speedup 1.0000x reference)
"""Trainium2 Bass kernel for the capsule-routing module.

Full-input contract: kernel(**inputs) takes the full [32,...] inputs,
shards batch over 8 NeuronCores (4 per core), runs the Bass kernel via
run_bass_kernel_spmd, and concatenates per-core outputs.

Math (per core, BL=4 local batches):
  Never materializes Wn or u_hat.  With G[n,(k,c)] = c_route[b,c,n] *
  alpha[n,c,k]:
    v[b,c,o]   = sum_{k,i} W[k,i,o] * hT[b][i,(k,c)],
                 hT[b][i,(k,c)] = sum_n x[b,n,i] * G[b][n,(k,c)]
    a[b,c,n]   = sum_k alpha[n,c,k] * e[b][(k,c),n],
                 e[b][(k,c),n] = sum_i wv[b][i,(k,c)] * xT[b][i,n]
                 wv[b][i,(k,c)] = sum_o W[k,i,o] * v[b,c,o]
  All five mm stages run with fp16 inputs / fp32 PSUM accumulation (1
  cyc/row on the PE vs fp32r's 2-4, and halved LDWEIGHTS + DMA bytes).
  With fp16 the routing-flip noise is ~1.6e-2, inside the 2e-2 gate,
  but ONLY if the squash factor sqrt(sn)/(1+sn) is near-exact: a 1e-4
  factor error alone costs ~1.4e-2 (the baseline's exp/ln-table chain
  was the dominant error).  Routing passes therefore refine the factor
  with Newton steps on the DVE (reciprocal_approx_accurate for
  1/(1+sn), one rsqrt-form Newton on the exp/ln seed), and the tiny
  fac-selector matmuls run in full fp32 so nothing re-quantizes it.
  Routing logits accumulate directly in a PSUM bank across passes
  (start at pass 0, stop at the last routing pass); softmax reads the
  running sums in place.  The final-pass output transposes the
  UNSCALED v and applies the (short exp/ln) factor as a per-partition
  scalar afterward, keeping the tail chain off the PE.
"""

import sys

sys.path.insert(0, "/opt/trn_rl_repo")

from contextlib import ExitStack

import numpy as np

import concourse.bacc as bacc
import concourse.mybir as mybir
import concourse.tile as tile

F32 = mybir.dt.float32
F16 = mybir.dt.float16
FR = mybir.dt.float32r
AX = mybir.AxisListType
ALU = mybir.AluOpType
ACTF = mybir.ActivationFunctionType

B, NODES, IN_DIM, OUT_DIM, CAPS, K, NUM_ROUTE = 32, 512, 256, 128, 16, 5, 3
NCORES = 8
BL = B // NCORES          # 4 batches per core
NCH = NODES // 128        # 4 node chunks
IH = IN_DIM // 128        # 2 input-dim chunks
Q = K * CAPS              # 80 = (k,c) packed, q = k*16 + c
NC10 = K * IH             # 10 contraction chunks over (k, ih)
NG = BL * NCH             # 16 softmax groups (b, nch)
BC = BL * CAPS            # 64


def caps_kernel(ctx, tc, out_d, x_d, xt_d, w2_d, w2t_d, a2g_d, g0_d,
                ae_d, ssel_d, ident_d, cpack_d):
    nc = tc.nc

    sb = ctx.enter_context(tc.tile_pool(name="sb", bufs=1))
    work = ctx.enter_context(tc.tile_pool(name="work", bufs=2))
    ps_log = ctx.enter_context(tc.tile_pool(name="ps_log", bufs=1, space="PSUM"))
    ps_h = ctx.enter_context(tc.tile_pool(name="ps_h", bufs=1, space="PSUM"))
    ps_e = ctx.enter_context(tc.tile_pool(name="ps_e", bufs=3, space="PSUM"))
    ps_wv = ctx.enter_context(tc.tile_pool(name="ps_wv", bufs=1, space="PSUM"))
    ps_s = ctx.enter_context(tc.tile_pool(name="ps_s", bufs=1, space="PSUM"))

    # ---------------- persistent SBUF ----------------
    ident = sb.tile([128, 128], F16, tag="ident")

    x_sb = sb.tile([128, NG * IN_DIM], F16, tag="x_sb")          # [p, (b,j,i)]
    xt_sb = sb.tile([128, BL * IH * NODES], F16, tag="xt_sb")    # [i, (b,ih,n)]
    w2 = sb.tile([128, NC10 * 128], F16, tag="w2")               # [i, (c10,o)]
    w2t = sb.tile([128, NC10 * 128], F16, tag="w2t")             # [o, (c10,ki)]
    a2g = sb.tile([128, NCH * Q], F32, tag="a2g")                # [p, (j,k,c)]
    a_e = sb.tile([Q, NODES], F32, tag="a_e")                    # [q, n]
    s_sel = sb.tile([Q, CAPS], F16, tag="s_sel")                 # [q, c]
    g0 = sb.tile([128, NCH * Q], F16, tag="g0")                  # iter-0 G
    cpack = sb.tile([128, 88], F32, tag="cpack")
    ones4 = cpack[:, 0:4]                                        # [128, 4]
    rc_sel = cpack[:BC, 4:84]                                    # [(b,c), q]
    fmask = cpack[:BC, 84:88]                                    # [(b,c), b]
    warm = sb.tile([1, 1], F32, tag="warm")
    # routing logits live in SBUF; per-pass increments are matmul'd into
    # a scratch PSUM bank then added on DVE (PSUM cannot accumulate
    # across closed matmul groups, and reads require closing the group)
    logits = sb.tile([128, NG * CAPS], F32, tag="logits")
    # one shared PSUM bank for all small matmul/transpose outputs
    # (f16 transposes write packed f16: bitcast views over f32 columns)
    small = ps_s.tile([128, 512], F32, tag="small")
    sm_htp = [small[:, 0:40].bitcast(F16), small[:, 40:80].bitcast(F16),
              small[:, 216:256].bitcast(F16), small[:, 256:296].bitcast(F16)]
    sm_vps = small[:, 80:80 + BC]
    sm_snq4 = small[:BC, 144:148]
    sm_snq = small[:BC, 144:145]
    sm_facq = small[:Q, 148:148 + BL]
    sm_outp = small[:BC, 152:152 + 64].bitcast(F16)

    # ---------------- input DMA ----------------
    # Two parallel HWDGE issue queues (Sync + Activation).  Pass-0
    # critical tensors first; one consolidated DMA per tensor (issue
    # cost on the queue is ~600 ns each).  The contribution input is
    # dropped: softmax over caps is invariant to the per-(b,n) constant.
    def xchunk(i):
        return (x_sb[:, i * 512:(i + 1) * 512], x_d[:, i * 512:(i + 1) * 512])

    nc.sync.dma_start(g0[:], g0_d[:, :])
    for i in (0, 1, 2, 3):           # batches 0-1
        nc.sync.dma_start(*xchunk(i))

    nc.scalar.dma_start(ident[:], ident_d[:, :])
    nc.scalar.dma_start(w2[:], w2_d[:, :])
    for i in (4, 5, 6, 7):           # batches 2-3
        nc.scalar.dma_start(*xchunk(i))
    # warm the ln/exp activation table between DMA issues
    nc.gpsimd.memset(warm[:1, :1], 1.0)
    nc.scalar.activation(warm[:1, :1], warm[:1, :1], ACTF.Ln)
    nc.scalar.dma_start(cpack[:], cpack_d[:, :])
    nc.scalar.dma_start(w2t[:], w2t_d[:, :])
    nc.scalar.dma_start(xt_sb[:], xt_d[:, :])
    nc.scalar.dma_start(a2g[:], a2g_d[:, :])
    nc.scalar.dma_start(a_e[:Q, :], ae_d[:, :])
    nc.scalar.dma_start(s_sel[:Q, :], ssel_d[:, :])
    nc.gpsimd.memset(logits[:], 0.0)

    # ---------------- helpers ----------------
    def spread_copy(idx, dst, src):
        if idx % 2 == 1:
            nc.scalar.copy(dst, src)
        else:
            nc.vector.tensor_copy(dst, src)

    def alloc_softmax():
        return {
            "mx": work.tile([128, NG], F32, tag="mx", name="mx"),
            "sub": work.tile([128, NG * CAPS], F32, tag="sub", name="sub"),
            "exp": work.tile([128, NG * CAPS], F32, tag="exp", name="exp"),
            "sm": work.tile([128, NG], F32, tag="sm", name="sm"),
            "rc": work.tile([128, NG], F32, tag="rc", name="rc"),
            "sn2": work.tile([128, NG * CAPS], F32, tag="sn2", name="sn2"),
            "gt": work.tile([128, NG * Q], F16, tag="gt", name="gt"),
        }

    def emit_softmax_b(b, s):
        # softmax over caps for one batch (reading the PSUM logit sums)
        # + fused G build: gt = (exp*rc) * a2g
        mx, sub, exp, sm, rc, sn2, gt = (s["mx"], s["sub"], s["exp"],
                                         s["sm"], s["rc"], s["sn2"], s["gt"])
        gs = slice(b * NCH, (b + 1) * NCH)
        cs = slice(b * NCH * CAPS, (b + 1) * NCH * CAPS)
        lg3 = logits[:, cs].rearrange("p (g c) -> p g c", g=NCH)
        nc.vector.reduce_max(mx[:, gs], lg3, axis=AX.X)
        nc.vector.tensor_sub(
            sub[:, cs].rearrange("p (g c) -> p g c", g=NCH),
            lg3,
            mx[:, gs].unsqueeze(2).broadcast_to([128, NCH, CAPS]),
        )
        nc.scalar.activation(exp[:, cs], sub[:, cs], ACTF.Exp)
        nc.vector.reduce_sum(
            sm[:, gs],
            exp[:, cs].rearrange("p (g c) -> p g c", g=NCH),
            axis=AX.X)
        nc.vector.reciprocal(rc[:, gs], sm[:, gs])
        nc.vector.tensor_mul(
            sn2[:, cs].rearrange("p (g c) -> p g c", g=NCH),
            exp[:, cs].rearrange("p (g c) -> p g c", g=NCH),
            rc[:, gs].unsqueeze(2).broadcast_to([128, NCH, CAPS]),
        )
        nc.gpsimd.tensor_mul(
            gt[:, b * NCH * Q:(b + 1) * NCH * Q]
            .rearrange("p (g k c) -> p g k c", g=NCH, k=K),
            sn2[:, cs].rearrange("p (g c) -> p g c", g=NCH)
            .unsqueeze(2).broadcast_to([128, NCH, K, CAPS]),
            a2g[:].rearrange("p (g k c) -> p g k c", g=NCH, k=K),
        )

    def fac_seed_chain(sncp, pfx):
        """exp/ln seed f0 = exp(0.5*ln(sn) - ln(1+sn)) on the scalar
        engine (runs concurrently with the DVE reciprocal chain)."""
        lnsn = work.tile([BC, 1], F32, tag=pfx + "lnsn")
        nc.scalar.activation(lnsn[:BC, :], sncp[:BC, :], ACTF.Ln)
        ln1p = work.tile([BC, 1], F32, tag=pfx + "ln1p")
        nc.scalar.activation(ln1p[:BC, :], sncp[:BC, :], ACTF.Ln, bias=1.0)
        arg = work.tile([BC, 1], F32, tag=pfx + "arg")
        nc.vector.scalar_tensor_tensor(arg[:BC, :], lnsn[:BC, :], 0.5,
                                       ln1p[:BC, :],
                                       op0=ALU.mult, op1=ALU.subtract)
        f0 = work.tile([BC, 1], F32, tag=pfx + "f0")
        nc.scalar.activation(f0[:BC, :], arg[:BC, :], ACTF.Exp)
        return f0

    # ---------------- routing ----------------
    cur = None   # softmax tiles for the current pass (None => uniform g0)
    for t in range(NUM_ROUTE + 1):
        fin = (t == NUM_ROUTE)
        if cur is None:
            def g_slice(b, j):
                return g0[:, j * Q:(j + 1) * Q]
        else:
            def g_slice(b, j, gt=cur["gt"]):
                return gt[:, (b * NCH + j) * Q:(b * NCH + j + 1) * Q]

        # --- h[b] = G_b^T @ x_b : psum [q(80) x i(256)] per b; all 16
        # --- h matmuls first, then all 8 PE transposes (keeps the PE
        # --- queue free of copy-stalls) ---
        ht_sb = work.tile([128, BL * IH * Q], F16, tag="ht")
        h_sbs = []
        for b in range(BL):
            hps = ps_h.tile([Q, IN_DIM], F32, tag="hps")
            for j in range(NCH):
                nc.tensor.matmul(
                    hps[:Q, :],
                    g_slice(b, j),
                    x_sb[:, (b * NCH + j) * IN_DIM:
                         (b * NCH + j + 1) * IN_DIM],
                    start=(j == 0),
                    stop=(j == NCH - 1),
                )
            h_sb = work.tile([Q, IN_DIM], F16, tag=f"h{b}")
            if b % 2 == 0:
                nc.scalar.copy(h_sb[:Q, :], hps[:Q, :])
            else:
                nc.vector.tensor_copy(h_sb[:Q, :], hps[:Q, :])
            h_sbs.append(h_sb)
        for b in range(BL):
            for ih in range(IH):
                htp = sm_htp[(b * IH + ih) % 4]
                nc.tensor.transpose(
                    htp,
                    h_sbs[b][:Q, ih * 128:(ih + 1) * 128],
                    ident[:Q, :Q],
                )
                spread_copy(b * IH + ih,
                            ht_sb[:, (b * IH + ih) * Q:(b * IH + ih + 1) * Q],
                            htp)

        # --- V[o, (b,c)] = sum_{k,i} W2[(ki),o] * hT[b][i,(k,c)] ---
        vps = sm_vps
        ht_v = ht_sb[:].rearrange("p (b ih q) -> p b ih q", b=BL, ih=IH)
        for c10 in range(NC10):
            k, ih = divmod(c10, IH)
            nc.tensor.matmul(
                vps.rearrange("p (b c) -> p b c", b=BL),
                w2[:, c10 * 128:(c10 + 1) * 128],
                ht_v[:, :, ih, k * CAPS:(k + 1) * CAPS],
                start=(c10 == 0),
                stop=(c10 == NC10 - 1),
            )
        v_sb = work.tile([128, BC], F16, tag="v_sb")
        nc.vector.tensor_copy(v_sb[:], vps)
        # sn = sum_o v^2 per (b,c), from the fp32 PSUM v (Square is in
        # every act table; also keeps the second PSUM read off the DVE)
        sq = work.tile([128, BC], F32, tag="sq")
        nc.scalar.activation(sq[:], vps, ACTF.Square)

        if fin:
            # transpose the UNSCALED v now (PE), scale by fac afterward
            # as a per-partition scalar
            outp = sm_outp
            nc.tensor.transpose(outp, v_sb[:], ident[:])
            nc.tensor.matmul(sm_snq4, sq[:], ones4, start=True, stop=True)
            sncp = work.tile([BC, 1], F32, tag="sncpf")
            nc.vector.tensor_copy(sncp[:BC, :], sm_snq)
            facx = fac_seed_chain(sncp, "fin_")
            out_sb = work.tile([BC, 128], F32, tag="outsb")
            nc.vector.tensor_scalar(out_sb[:BC, :], outp, facx[:BC, 0:1],
                                    None, op0=ALU.mult)
            nc.sync.dma_start(
                out_d.rearrange("b c o -> (b c) o"),
                out_sb[:BC, :],
            )
            break

        # --- squash factor fac = sqrt(sn)/(1+sn), Newton-refined:
        # ---   r  = 1/(1+sn)   (reciprocal_approx_accurate, ~2 ulp)
        # ---   u  = sn*r^2     (= fac^2)
        # ---   y  = 1/sqrt(u)  (seed 1/f0 from exp/ln chain + 1 Newton)
        # ---   fac = u*y
        # The Newton products run on gpsimd so the DVE queue stays free
        # for the wv spreads; the tiny snq4 matmul is emitted before wv
        # so the chain starts at v-end and hides under wv+e.
        nc.tensor.matmul(sm_snq4, sq[:], ones4, start=True, stop=True)
        sncp = work.tile([BC, 1], F32, tag="sncp")
        nc.vector.tensor_copy(sncp[:BC, :], sm_snq)
        f0 = fac_seed_chain(sncp, "rt_")
        ap1 = work.tile([BC, 1], F32, tag="ap1")
        nc.vector.tensor_scalar(ap1[:BC, :], sncp[:BC, :], 1.0, None,
                                op0=ALU.add)
        rscr = work.tile([BC, 1], F32, tag="rscr")
        rr = work.tile([BC, 1], F32, tag="rr")
        nc.vector.reciprocal_approx_accurate(rr[:BC, :], ap1[:BC, :],
                                             rscr[:BC, :])
        y0 = work.tile([BC, 1], F32, tag="y0")
        nc.vector.reciprocal_approx_fast(y0[:BC, :], f0[:BC, :])

        # --- wv[i, (k,b,c)] = sum_o W[k,i,o] * v[o, (b,c)] (unscaled);
        # --- the DVE Newton ops interleave with the wv spread copies so
        # --- the fac chain and e-feeding both progress ---
        wvp = ps_wv.tile([128, NC10 * BC], F32, tag="wvp")
        for c10 in range(NC10):
            nc.tensor.matmul(
                wvp[:, c10 * BC:(c10 + 1) * BC],
                w2t[:, c10 * 128:(c10 + 1) * 128],
                v_sb[:],
                start=True, stop=True,
            )
        wv_sb = work.tile([128, IH * BL * Q], F16, tag="wv")
        wvp_v = wvp[:].rearrange("p (k ih b c) -> p ih b k c",
                                 k=K, ih=IH, b=BL)

        def wv_spread(b, ih):
            spread_copy(b * IH + ih,
                        wv_sb[:, (ih * BL + b) * Q:(ih * BL + b + 1) * Q]
                        .rearrange("p (k c) -> p k c", k=K),
                        wvp_v[:, ih, b])

        for bb in range(BL):
            wv_spread(bb, 0)
            wv_spread(bb, 1)
        u1 = work.tile([BC, 1], F32, tag="u1")
        nc.vector.tensor_mul(u1[:BC, :], sncp[:BC, :], rr[:BC, :])
        uu = work.tile([BC, 1], F32, tag="uu")
        nc.vector.tensor_mul(uu[:BC, :], u1[:BC, :], rr[:BC, :])
        z1 = work.tile([BC, 1], F32, tag="z1")
        nc.vector.tensor_mul(z1[:BC, :], uu[:BC, :], y0[:BC, :])
        z2 = work.tile([BC, 1], F32, tag="z2")
        nc.vector.tensor_mul(z2[:BC, :], z1[:BC, :], y0[:BC, :])
        wn = work.tile([BC, 1], F32, tag="wn")
        nc.vector.tensor_scalar(wn[:BC, :], z2[:BC, :], -0.5, 1.5,
                                op0=ALU.mult, op1=ALU.add)
        y1 = work.tile([BC, 1], F32, tag="y1")
        nc.vector.tensor_mul(y1[:BC, :], y0[:BC, :], wn[:BC, :])
        facx = work.tile([BC, 1], F32, tag="facx")
        nc.vector.tensor_mul(facx[:BC, :], uu[:BC, :], y1[:BC, :])
        # facq[q, b] = fac[b, c(q)] via a constant fp32 selector matmul;
        # the matmul itself is emitted inside the e pipeline (after
        # e(b1)) so the fac chain never blocks the PE
        rhsm = work.tile([BC, BL], F32, tag="rhsm")
        nc.vector.tensor_mul(rhsm[:BC, :],
                             facx[:BC, 0:1].broadcast_to([BC, BL]),
                             fmask)
        facq = work.tile([Q, BL], F32, tag="facqs")

        # --- e[b] = wv_b^T @ xT_b : [q(80) x n(512)];
        # --- tmp = e * fac[b,c(q)] * alpha; logit mms accumulate into
        # --- the persistent aps PSUM regions; then immediately emit the
        # --- NEXT pass's softmax for this b ---
        aps = ps_log.tile([128, NG * CAPS], F32, tag="aps")
        nxt = alloc_softmax()
        epss = {}

        def emit_e(b):
            eps = ps_e.tile([Q, NODES], F32, tag="eps")
            for ih in range(IH):
                nc.tensor.matmul(
                    eps[:Q, :],
                    wv_sb[:, (ih * BL + b) * Q:(ih * BL + b + 1) * Q],
                    xt_sb[:, (b * IH + ih) * NODES:
                          (b * IH + ih + 1) * NODES],
                    start=(ih == 0),
                    stop=(ih == IH - 1),
                )
            epss[b] = eps

        def emit_tmp_log(b):
            eps = epss[b]
            tmp = work.tile([Q, NODES], F16, tag="tmp")
            # halved so the first logit matmuls start after half an STT
            for hs, js in ((slice(0, 256), (0, 1)),
                           (slice(256, 512), (2, 3))):
                nc.vector.scalar_tensor_tensor(tmp[:Q, hs], eps[:Q, hs],
                                               facq[:Q, b:b + 1],
                                               a_e[:Q, hs],
                                               op0=ALU.mult, op1=ALU.mult)
                for j in js:
                    g = b * NCH + j
                    nc.tensor.matmul(
                        aps[:, g * CAPS:(g + 1) * CAPS],
                        tmp[:Q, j * 128:(j + 1) * 128],
                        s_sel[:Q, :],
                        start=True, stop=True,
                    )
            bs = slice(b * NCH * CAPS, (b + 1) * NCH * CAPS)
            nc.vector.tensor_add(logits[:, bs], logits[:, bs], aps[:, bs])
            emit_softmax_b(b, nxt)

        emit_e(0)
        emit_e(1)
        emit_e(2)
        nc.tensor.matmul(sm_facq, rc_sel, rhsm[:BC, :],
                         start=True, stop=True)
        nc.vector.tensor_copy(facq[:Q, :], sm_facq)
        emit_tmp_log(0)
        emit_e(3)
        emit_tmp_log(1)
        emit_tmp_log(2)
        emit_tmp_log(3)
        cur = nxt


_CACHE = {}


def _force_combined_exp_ln_table(arch):
    """Make natural_log_exp_and_others the only act set offering Exp/Ln so
    the table-load pass never alternates tables between softmax (Exp) and
    the squash-factor seed (Ln)."""
    from concourse.hw_specs import get_activation_tables
    try:
        tabs = get_activation_tables(arch)
    except Exception:
        return
    for name, s in tabs.items():
        if name != "natural_log_exp_and_others":
            s.discard(ACTF.Exp)
            s.discard(ACTF.Ln)


def _build():
    if "nc" in _CACHE:
        return _CACHE["nc"]
    nc = bacc.Bacc("TRN2", target_bir_lowering=False, debug=False,
                   num_devices=NCORES)
    _force_combined_exp_ln_table(nc.m.arch)
    x_d = nc.dram_tensor("x", [128, NG * IN_DIM], F16, kind="ExternalInput")
    xt_d = nc.dram_tensor("xt", [128, BL * IH * NODES], F16,
                          kind="ExternalInput")
    w2_d = nc.dram_tensor("w2", [128, NC10 * 128], F16, kind="ExternalInput")
    w2t_d = nc.dram_tensor("w2t", [128, NC10 * 128], F16,
                           kind="ExternalInput")
    a2g_d = nc.dram_tensor("a2g", [128, NCH * Q], F32, kind="ExternalInput")
    g0_d = nc.dram_tensor("g0", [128, NCH * Q], F16, kind="ExternalInput")
    ae_d = nc.dram_tensor("a_e", [Q, NODES], F32, kind="ExternalInput")
    ssel_d = nc.dram_tensor("s_sel", [Q, CAPS], F16, kind="ExternalInput")
    ident_d = nc.dram_tensor("ident", [128, 128], F16, kind="ExternalInput")
    cpack_d = nc.dram_tensor("cpack", [128, 88], F32, kind="ExternalInput")
    out_d = nc.dram_tensor("out", [BL, CAPS, OUT_DIM], F32,
                           kind="ExternalOutput")
    with tile.TileContext(nc) as tc:
        with ExitStack() as ctx:
            caps_kernel(ctx, tc, out_d.ap(), x_d.ap(),
                        xt_d.ap(), w2_d.ap(), w2t_d.ap(), a2g_d.ap(),
                        g0_d.ap(), ae_d.ap(), ssel_d.ap(), ident_d.ap(),
                        cpack_d.ap())
    nc.compile()
    _CACHE["nc"] = nc
    return nc


def host_prep(W, alpha):
    """Constant input layouts shared by all cores."""
    W = np.asarray(W, dtype=np.float32)
    alpha = np.asarray(alpha, dtype=np.float32)
    w2 = np.ascontiguousarray(
        W.reshape(K, IH, 128, OUT_DIM).transpose(2, 0, 1, 3)
        .reshape(128, NC10 * 128)).astype(np.float16)
    w2t = np.ascontiguousarray(
        W.reshape(K, IH, 128, OUT_DIM).transpose(3, 0, 1, 2)
        .reshape(128, NC10 * 128)).astype(np.float16)
    a2g = np.ascontiguousarray(
        alpha.reshape(NCH, 128, CAPS, K).transpose(1, 0, 3, 2)
        .reshape(128, NCH * Q))
    a_e = np.ascontiguousarray(
        alpha.transpose(2, 1, 0).reshape(Q, NODES))
    s_sel = np.ascontiguousarray(
        np.tile(np.eye(CAPS, dtype=np.float32), (K, 1))).astype(np.float16)
    ident = np.eye(128, dtype=np.float32).astype(np.float16)
    # cpack: ones4 | rc_sel[(b,c),q]=[c==q%CAPS] | fmask[(b,c),b]=[bc//CAPS==b]
    cpack = np.zeros((128, 88), dtype=np.float32)
    cpack[:, 0:4] = 1.0
    cidx = np.arange(BC) % CAPS
    cpack[:BC, 4:84] = (cidx[:, None] == (np.arange(Q) % CAPS)[None, :])
    bidx = np.arange(BC) // CAPS
    cpack[:BC, 84:88] = (bidx[:, None] == np.arange(BL)[None, :])
    g0 = np.ascontiguousarray(a2g * (1.0 / CAPS)).astype(np.float16)
    return {"w2": w2, "w2t": w2t, "a2g": a2g, "g0": g0, "a_e": a_e,
            "s_sel": s_sel, "ident": ident, "cpack": cpack}


def prep_xt(xl):
    """Per-core xT layout [i_local(128), (b, ih, n)]."""
    return np.ascontiguousarray(
        xl.reshape(BL, NODES, IH, 128).transpose(3, 0, 2, 1)
        .reshape(128, BL * IH * NODES))


def make_in_maps(x, W, alpha):
    consts = host_prep(W, alpha)
    in_maps = []
    for c in range(NCORES):
        xl = np.asarray(x, dtype=np.float32)[c * BL:(c + 1) * BL] \
            .astype(np.float16)
        xp = np.ascontiguousarray(
            xl.reshape(BL, NCH, 128, IN_DIM).transpose(2, 0, 1, 3)
            .reshape(128, NG * IN_DIM))
        in_maps.append({"x": xp, "xt": prep_xt(xl), **consts})
    return in_maps


def _enable_ldw_opt():
    # no-op: walrus' LDW optimization rejects the standalone InstLdweights
    # that 16-bit matmuls emit ("InstLdweights is not compatible with LDW
    # optimization"); the fp32r baseline kept it on, but it never deduped
    # anything there either.
    return


def kernel(x, contribution, W, alpha):
    from concourse import bass_utils
    _enable_ldw_opt()

    nc = _build()
    in_maps = make_in_maps(x, W, alpha)
    res = bass_utils.run_bass_kernel_spmd(nc, in_maps,
                                          core_ids=list(range(NCORES)))
    return np.concatenate([res.results[c]["out"] for c in range(NCORES)],
                          axis=0)


# revision 17
# speedup vs baseline: 1.2507x; 1.2507x over previous
"""Trainium2 Bass kernel for the capsule-routing module.

Full-input contract: kernel(**inputs) takes the full [32,...] inputs,
shards batch over 8 NeuronCores (4 per core), runs the Bass kernel via
run_bass_kernel_spmd, and concatenates per-core outputs.

Math (per core, BL=4 local batches):
  Never materializes Wn or u_hat.  With G[n,(k,c)] = c_route[b,c,n] *
  alpha[n,c,k]:
    v[b,c,o]   = sum_{k,i} W[k,i,o] * hT[b][i,(k,c)],
                 hT[b][i,(k,c)] = sum_n x[b,n,i] * G[b][n,(k,c)]
    a[b,c,n]   = sum_k alpha[n,c,k] * e[b][(k,c),n],
                 e[b][(k,c),n] = sum_i wv[b][i,(k,c)] * xT[b][i,n]
                 wv[b][i,(k,c)] = sum_o W[k,i,o] * v[b,c,o]
  All five mm stages run with fp16 inputs / fp32 PSUM accumulation (1
  cyc/row on the PE vs fp32r's 2-4, and halved LDWEIGHTS + DMA bytes).
  With fp16 the routing-flip noise is ~1.6e-2, inside the 2e-2 gate,
  but ONLY if the squash factor sqrt(sn)/(1+sn) is near-exact: a 1e-4
  factor error alone costs ~1.4e-2 (the baseline's exp/ln-table chain
  was the dominant error).  Routing passes therefore refine the factor
  with Newton steps on the DVE (reciprocal_approx_accurate for
  1/(1+sn), one rsqrt-form Newton on the exp/ln seed), and the tiny
  fac-selector matmuls run in full fp32 so nothing re-quantizes it.
  Routing logits accumulate directly in a PSUM bank across passes
  (start at pass 0, stop at the last routing pass); softmax reads the
  running sums in place.  The final-pass output transposes the
  UNSCALED v and applies the (short exp/ln) factor as a per-partition
  scalar afterward, keeping the tail chain off the PE.
"""

import sys

sys.path.insert(0, "/opt/trn_rl_repo")

from contextlib import ExitStack

import numpy as np

import concourse.bacc as bacc
import concourse.mybir as mybir
import concourse.tile as tile

F32 = mybir.dt.float32
F16 = mybir.dt.float16
FR = mybir.dt.float32r
AX = mybir.AxisListType
ALU = mybir.AluOpType
ACTF = mybir.ActivationFunctionType

B, NODES, IN_DIM, OUT_DIM, CAPS, K, NUM_ROUTE = 32, 512, 256, 128, 16, 5, 3
NCORES = 8
BL = B // NCORES          # 4 batches per core
NCH = NODES // 128        # 4 node chunks
IH = IN_DIM // 128        # 2 input-dim chunks
Q = K * CAPS              # 80 = (k,c) packed, q = k*16 + c
NC10 = K * IH             # 10 contraction chunks over (k, ih)
NG = BL * NCH             # 16 softmax groups (b, nch)
BC = BL * CAPS            # 64


def caps_kernel(ctx, tc, out_d, x_d, xt_d, w2_d, w2t_d, a2g_d, g0_d,
                ae_d, ssel_d, ident_d, cpack_d):
    nc = tc.nc

    sb = ctx.enter_context(tc.tile_pool(name="sb", bufs=1))
    work = ctx.enter_context(tc.tile_pool(name="work", bufs=2))
    ps_log = ctx.enter_context(tc.tile_pool(name="ps_log", bufs=1, space="PSUM"))
    ps_h = ctx.enter_context(tc.tile_pool(name="ps_h", bufs=2, space="PSUM"))
    ps_e = ctx.enter_context(tc.tile_pool(name="ps_e", bufs=2, space="PSUM"))
    ps_wv = ctx.enter_context(tc.tile_pool(name="ps_wv", bufs=1, space="PSUM"))
    ps_s = ctx.enter_context(tc.tile_pool(name="ps_s", bufs=1, space="PSUM"))

    # ---------------- persistent SBUF ----------------
    ident = sb.tile([128, 128], F16, tag="ident")

    x_sb = sb.tile([128, NG * IN_DIM], F16, tag="x_sb")          # [p, (b,j,i)]
    xt_sb = sb.tile([128, BL * IH * NODES], F16, tag="xt_sb")    # [i, (b,ih,n)]
    w2 = sb.tile([128, NC10 * 128], F16, tag="w2")               # [i, (c10,o)]
    w2t = sb.tile([128, NC10 * 128], F16, tag="w2t")             # [o, (c10,ki)]
    a2g = sb.tile([128, NCH * Q], F32, tag="a2g")                # [p, (j,k,c)]
    a_e = sb.tile([Q, NODES], F32, tag="a_e")                    # [q, n]
    s_sel = sb.tile([Q, CAPS], F16, tag="s_sel")                 # [q, c]
    g0 = sb.tile([128, NCH * Q], F16, tag="g0")                  # iter-0 G
    cpack = sb.tile([128, 88], F32, tag="cpack")
    ones4 = cpack[:, 0:4]                                        # [128, 4]
    rc_sel = cpack[:BC, 4:84]                                    # [(b,c), q]
    fmask = cpack[:BC, 84:88]                                    # [(b,c), b]
    warm = sb.tile([1, 1], F32, tag="warm")
    # routing logits live in SBUF; per-pass increments are matmul'd into
    # a scratch PSUM bank then added on DVE (PSUM cannot accumulate
    # across closed matmul groups, and reads require closing the group)
    logits = sb.tile([128, NG * CAPS], F32, tag="logits")
    # one shared PSUM bank for all small matmul/transpose outputs
    # (f16 transposes write packed f16: bitcast views over f32 columns)
    small = ps_s.tile([128, 512], F32, tag="small")
    sm_htp = [small[:, 0:40].bitcast(F16), small[:, 40:80].bitcast(F16),
              small[:, 216:256].bitcast(F16), small[:, 256:296].bitcast(F16),
              small[:, 296:336].bitcast(F16), small[:, 336:376].bitcast(F16),
              small[:, 376:416].bitcast(F16), small[:, 416:456].bitcast(F16)]
    sm_vps = small[:, 80:80 + BC]
    sm_snq4 = small[:BC, 144:148]
    sm_snq = small[:BC, 144:145]
    sm_facq = small[:Q, 148:148 + BL]
    sm_outp = small[:BC, 152:152 + 64].bitcast(F16)

    # ---------------- input DMA ----------------
    # Two parallel HWDGE issue queues (Sync + Activation).  Pass-0
    # critical tensors first; one consolidated DMA per tensor (issue
    # cost on the queue is ~600 ns each).  The contribution input is
    # dropped: softmax over caps is invariant to the per-(b,n) constant.
    def xchunk(b):
        return (x_sb[:, b * 1024:(b + 1) * 1024],
                x_d[:, b * 1024:(b + 1) * 1024])

    nc.sync.dma_start(g0[:], g0_d[:, :])
    nc.sync.dma_start(*xchunk(0))
    nc.sync.dma_start(*xchunk(1))

    nc.scalar.dma_start(ident[:], ident_d[:, :])
    nc.scalar.dma_start(*xchunk(2))
    nc.scalar.dma_start(w2[:], w2_d[:, :])
    nc.scalar.dma_start(*xchunk(3))
    # warm the ln/exp activation table between DMA issues
    nc.gpsimd.memset(warm[:1, :1], 1.0)
    nc.scalar.activation(warm[:1, :1], warm[:1, :1], ACTF.Ln)
    nc.scalar.dma_start(cpack[:], cpack_d[:, :])
    nc.scalar.dma_start(w2t[:], w2t_d[:, :])
    nc.scalar.dma_start(xt_sb[:], xt_d[:, :])
    nc.scalar.dma_start(a2g[:], a2g_d[:, :])
    nc.scalar.dma_start(a_e[:Q, :], ae_d[:, :])
    nc.scalar.dma_start(s_sel[:Q, :], ssel_d[:, :])
    nc.gpsimd.memset(logits[:], 0.0)

    # ---------------- helpers ----------------
    def spread_copy(idx, dst, src):
        if idx % 2 == 1:
            nc.scalar.copy(dst, src)
        else:
            nc.vector.tensor_copy(dst, src)

    def alloc_softmax():
        return {
            "mx": work.tile([128, NG], F32, tag="mx", name="mx"),
            "sub": work.tile([128, NG * CAPS], F32, tag="sub", name="sub"),
            "exp": work.tile([128, NG * CAPS], F32, tag="exp", name="exp"),
            "sm": work.tile([128, NG], F32, tag="sm", name="sm"),
            "rc": work.tile([128, NG], F32, tag="rc", name="rc"),
            "sn2": work.tile([128, NG * CAPS], F32, tag="sn2", name="sn2"),
            "gt": work.tile([128, NG * Q], F16, tag="gt", name="gt"),
        }

    def emit_softmax_b(b, s):
        # softmax over caps for one batch (reading the PSUM logit sums)
        # + fused G build: gt = (exp*rc) * a2g
        mx, sub, exp, sm, rc, sn2, gt = (s["mx"], s["sub"], s["exp"],
                                         s["sm"], s["rc"], s["sn2"], s["gt"])
        gs = slice(b * NCH, (b + 1) * NCH)
        cs = slice(b * NCH * CAPS, (b + 1) * NCH * CAPS)
        lg3 = logits[:, cs].rearrange("p (g c) -> p g c", g=NCH)
        nc.vector.reduce_max(mx[:, gs], lg3, axis=AX.X)
        nc.vector.tensor_sub(
            sub[:, cs].rearrange("p (g c) -> p g c", g=NCH),
            lg3,
            mx[:, gs].unsqueeze(2).broadcast_to([128, NCH, CAPS]),
        )
        nc.scalar.activation(exp[:, cs], sub[:, cs], ACTF.Exp)
        nc.vector.reduce_sum(
            sm[:, gs],
            exp[:, cs].rearrange("p (g c) -> p g c", g=NCH),
            axis=AX.X)
        nc.vector.reciprocal(rc[:, gs], sm[:, gs])
        nc.vector.tensor_mul(
            sn2[:, cs].rearrange("p (g c) -> p g c", g=NCH),
            exp[:, cs].rearrange("p (g c) -> p g c", g=NCH),
            rc[:, gs].unsqueeze(2).broadcast_to([128, NCH, CAPS]),
        )
        nc.gpsimd.tensor_mul(
            gt[:, b * NCH * Q:(b + 1) * NCH * Q]
            .rearrange("p (g k c) -> p g k c", g=NCH, k=K),
            sn2[:, cs].rearrange("p (g c) -> p g c", g=NCH)
            .unsqueeze(2).broadcast_to([128, NCH, K, CAPS]),
            a2g[:].rearrange("p (g k c) -> p g k c", g=NCH, k=K),
        )

    def fac_seed_chain(sncp, pfx):
        """exp/ln seed f0 = exp(0.5*ln(sn) - ln(1+sn)) on the scalar
        engine (runs concurrently with the DVE reciprocal chain)."""
        lnsn = work.tile([BC, 1], F32, tag=pfx + "lnsn")
        nc.scalar.activation(lnsn[:BC, :], sncp[:BC, :], ACTF.Ln)
        ln1p = work.tile([BC, 1], F32, tag=pfx + "ln1p")
        nc.scalar.activation(ln1p[:BC, :], sncp[:BC, :], ACTF.Ln, bias=1.0)
        arg = work.tile([BC, 1], F32, tag=pfx + "arg")
        nc.vector.scalar_tensor_tensor(arg[:BC, :], lnsn[:BC, :], 0.5,
                                       ln1p[:BC, :],
                                       op0=ALU.mult, op1=ALU.subtract)
        f0 = work.tile([BC, 1], F32, tag=pfx + "f0")
        nc.scalar.activation(f0[:BC, :], arg[:BC, :], ACTF.Exp)
        return f0

    # ---------------- routing ----------------
    cur = None   # softmax tiles for the current pass (None => uniform g0)
    for t in range(NUM_ROUTE + 1):
        fin = (t == NUM_ROUTE)
        if cur is None:
            def g_slice(b, j):
                return g0[:, j * Q:(j + 1) * Q]
        else:
            def g_slice(b, j, gt=cur["gt"]):
                return gt[:, (b * NCH + j) * Q:(b * NCH + j + 1) * Q]

        # --- h[b] = G_b^T @ x_b : psum [q(80) x i(256)] per b; all 16
        # --- h matmuls first, then all 8 PE transposes (keeps the PE
        # --- queue free of copy-stalls) ---
        ht_sb = work.tile([128, BL * IH * Q], F16, tag="ht")
        h_sbs = []
        for b in range(BL):
            hps = ps_h.tile([Q, IN_DIM], F32, tag="hps")
            for j in range(NCH):
                nc.tensor.matmul(
                    hps[:Q, :],
                    g_slice(b, j),
                    x_sb[:, (b * NCH + j) * IN_DIM:
                         (b * NCH + j + 1) * IN_DIM],
                    start=(j == 0),
                    stop=(j == NCH - 1),
                )
            h_sb = work.tile([Q, IN_DIM], F16, tag=f"h{b}")
            if b % 2 == 0:
                nc.scalar.copy(h_sb[:Q, :], hps[:Q, :])
            else:
                nc.vector.tensor_copy(h_sb[:Q, :], hps[:Q, :])
            h_sbs.append(h_sb)
        for b in range(BL):
            for ih in range(IH):
                htp = sm_htp[b * IH + ih]
                nc.tensor.transpose(
                    htp,
                    h_sbs[b][:Q, ih * 128:(ih + 1) * 128],
                    ident[:Q, :Q],
                )
                nc.vector.tensor_copy(
                    ht_sb[:, (b * IH + ih) * Q:(b * IH + ih + 1) * Q], htp)

        # --- V[o, (b,c)] = sum_{k,i} W2[(ki),o] * hT[b][i,(k,c)] ---
        vps = sm_vps
        ht_v = ht_sb[:].rearrange("p (b ih q) -> p b ih q", b=BL, ih=IH)
        for c10 in range(NC10):
            k, ih = divmod(c10, IH)
            nc.tensor.matmul(
                vps.rearrange("p (b c) -> p b c", b=BL),
                w2[:, c10 * 128:(c10 + 1) * 128],
                ht_v[:, :, ih, k * CAPS:(k + 1) * CAPS],
                start=(c10 == 0),
                stop=(c10 == NC10 - 1),
            )
        v_sb = work.tile([128, BC], F16, tag="v_sb")
        nc.vector.tensor_copy(v_sb[:], vps)
        # sn = sum_o v^2 per (b,c), from the fp32 PSUM v (Square is in
        # every act table; also keeps the second PSUM read off the DVE)
        sq = work.tile([128, BC], F32, tag="sq")
        nc.scalar.activation(sq[:], vps, ACTF.Square)

        if fin:
            # transpose the UNSCALED v now (PE), scale by fac afterward
            # as a per-partition scalar
            outp = sm_outp
            nc.tensor.transpose(outp, v_sb[:], ident[:])
            nc.tensor.matmul(sm_snq4, sq[:], ones4, start=True, stop=True)
            sncp = work.tile([BC, 1], F32, tag="sncpf")
            nc.vector.tensor_copy(sncp[:BC, :], sm_snq)
            facx = fac_seed_chain(sncp, "fin_")
            out_sb = work.tile([BC, 128], F32, tag="outsb")
            nc.vector.tensor_scalar(out_sb[:BC, :], outp, facx[:BC, 0:1],
                                    None, op0=ALU.mult)
            nc.sync.dma_start(
                out_d.rearrange("b c o -> (b c) o"),
                out_sb[:BC, :],
            )
            break

        # --- squash factor fac = sqrt(sn)/(1+sn), Newton-refined:
        # ---   r  = 1/(1+sn)   (reciprocal_approx_accurate, ~2 ulp)
        # ---   u  = sn*r^2     (= fac^2)
        # ---   y  = 1/sqrt(u)  (seed 1/f0 from exp/ln chain + 1 Newton)
        # ---   fac = u*y
        # The Newton products run on gpsimd so the DVE queue stays free
        # for the wv spreads; the tiny snq4 matmul is emitted before wv
        # so the chain starts at v-end and hides under wv+e.
        nc.tensor.matmul(sm_snq4, sq[:], ones4, start=True, stop=True)
        sncp = work.tile([BC, 1], F32, tag="sncp")
        nc.vector.tensor_copy(sncp[:BC, :], sm_snq)
        f0 = fac_seed_chain(sncp, "rt_")
        ap1 = work.tile([BC, 1], F32, tag="ap1")
        nc.vector.tensor_scalar(ap1[:BC, :], sncp[:BC, :], 1.0, None,
                                op0=ALU.add)
        rscr = work.tile([BC, 1], F32, tag="rscr")
        rr = work.tile([BC, 1], F32, tag="rr")
        nc.vector.reciprocal_approx_accurate(rr[:BC, :], ap1[:BC, :],
                                             rscr[:BC, :])
        y0 = work.tile([BC, 1], F32, tag="y0")
        nc.vector.reciprocal_approx_fast(y0[:BC, :], f0[:BC, :])

        # --- wv[i, (k,b,c)] = sum_o W[k,i,o] * v[o, (b,c)] (unscaled);
        # --- the DVE Newton ops interleave with the wv spread copies so
        # --- the fac chain and e-feeding both progress ---
        wvp = ps_wv.tile([128, NC10 * BC], F32, tag="wvp")
        for c10 in range(NC10):
            nc.tensor.matmul(
                wvp[:, c10 * BC:(c10 + 1) * BC],
                w2t[:, c10 * 128:(c10 + 1) * 128],
                v_sb[:],
                start=True, stop=True,
            )
        wv_sb = work.tile([128, IH * BL * Q], F16, tag="wv")
        wvp_v = wvp[:].rearrange("p (k ih b c) -> p ih b k c",
                                 k=K, ih=IH, b=BL)

        def wv_spread(b, ih):
            spread_copy(b * IH + ih,
                        wv_sb[:, (ih * BL + b) * Q:(ih * BL + b + 1) * Q]
                        .rearrange("p (k c) -> p k c", k=K),
                        wvp_v[:, ih, b])

        for bb in range(BL):
            wv_spread(bb, 0)
            wv_spread(bb, 1)
        u1 = work.tile([BC, 1], F32, tag="u1")
        nc.vector.tensor_mul(u1[:BC, :], sncp[:BC, :], rr[:BC, :])
        uu = work.tile([BC, 1], F32, tag="uu")
        nc.vector.tensor_mul(uu[:BC, :], u1[:BC, :], rr[:BC, :])
        z1 = work.tile([BC, 1], F32, tag="z1")
        nc.vector.tensor_mul(z1[:BC, :], uu[:BC, :], y0[:BC, :])
        z2 = work.tile([BC, 1], F32, tag="z2")
        nc.vector.tensor_mul(z2[:BC, :], z1[:BC, :], y0[:BC, :])
        wn = work.tile([BC, 1], F32, tag="wn")
        nc.vector.tensor_scalar(wn[:BC, :], z2[:BC, :], -0.5, 1.5,
                                op0=ALU.mult, op1=ALU.add)
        y1 = work.tile([BC, 1], F32, tag="y1")
        nc.vector.tensor_mul(y1[:BC, :], y0[:BC, :], wn[:BC, :])
        facx = work.tile([BC, 1], F32, tag="facx")
        nc.vector.tensor_mul(facx[:BC, :], uu[:BC, :], y1[:BC, :])
        # facq[q, b] = fac[b, c(q)] via a constant fp32 selector matmul;
        # the matmul itself is emitted inside the e pipeline (after
        # e(b1)) so the fac chain never blocks the PE
        rhsm = work.tile([BC, BL], F32, tag="rhsm")
        nc.vector.tensor_mul(rhsm[:BC, :],
                             facx[:BC, 0:1].broadcast_to([BC, BL]),
                             fmask)
        facq = work.tile([Q, BL], F32, tag="facqs")

        # --- e[b] = wv_b^T @ xT_b : [q(80) x n(512)];
        # --- tmp = e * fac[b,c(q)] * alpha; logit mms accumulate into
        # --- the persistent aps PSUM regions; then immediately emit the
        # --- NEXT pass's softmax for this b ---
        aps = ps_log.tile([128, NG * CAPS], F32, tag="aps")
        nxt = alloc_softmax()
        epss = {}

        def emit_e(b):
            eps = ps_e.tile([Q, NODES], F32, tag="eps")
            for ih in range(IH):
                nc.tensor.matmul(
                    eps[:Q, :],
                    wv_sb[:, (ih * BL + b) * Q:(ih * BL + b + 1) * Q],
                    xt_sb[:, (b * IH + ih) * NODES:
                          (b * IH + ih + 1) * NODES],
                    start=(ih == 0),
                    stop=(ih == IH - 1),
                )
            epss[b] = eps

        def emit_tmp_log(b):
            # tmp = (e * fac) * alpha: per-partition fac scale on the Act
            # engine (reads PSUM), alpha multiply on gpsimd -- keeps the
            # DVE free for softmax/reduces
            eps = epss[b]
            tmp_s = work.tile([Q, NODES], F32, tag="tmp_s")
            nc.scalar.activation(tmp_s[:Q, :], eps[:Q, :], ACTF.Copy,
                                 scale=facq[:Q, b:b + 1])
            tmp = work.tile([Q, NODES], F16, tag="tmp")
            nc.gpsimd.tensor_mul(tmp[:Q, :], tmp_s[:Q, :], a_e[:Q, :])
            for j in range(NCH):
                g = b * NCH + j
                nc.tensor.matmul(
                    aps[:, g * CAPS:(g + 1) * CAPS],
                    tmp[:Q, j * 128:(j + 1) * 128],
                    s_sel[:Q, :],
                    start=True, stop=True,
                )
            bs = slice(b * NCH * CAPS, (b + 1) * NCH * CAPS)
            nc.vector.tensor_add(logits[:, bs], logits[:, bs], aps[:, bs])
            emit_softmax_b(b, nxt)

        emit_e(0)
        emit_e(1)
        nc.tensor.matmul(sm_facq, rc_sel, rhsm[:BC, :],
                         start=True, stop=True)
        nc.vector.tensor_copy(facq[:Q, :], sm_facq)
        emit_tmp_log(0)
        emit_e(2)
        emit_tmp_log(1)
        emit_e(3)
        emit_tmp_log(2)
        emit_tmp_log(3)
        cur = nxt


_CACHE = {}


def _force_combined_exp_ln_table(arch):
    """Make natural_log_exp_and_others the only act set offering Exp/Ln so
    the table-load pass never alternates tables between softmax (Exp) and
    the squash-factor seed (Ln)."""
    from concourse.hw_specs import get_activation_tables
    try:
        tabs = get_activation_tables(arch)
    except Exception:
        return
    for name, s in tabs.items():
        if name != "natural_log_exp_and_others":
            s.discard(ACTF.Exp)
            s.discard(ACTF.Ln)


def _build():
    if "nc" in _CACHE:
        return _CACHE["nc"]
    nc = bacc.Bacc("TRN2", target_bir_lowering=False, debug=False,
                   num_devices=NCORES)
    _force_combined_exp_ln_table(nc.m.arch)
    x_d = nc.dram_tensor("x", [128, NG * IN_DIM], F16, kind="ExternalInput")
    xt_d = nc.dram_tensor("xt", [128, BL * IH * NODES], F16,
                          kind="ExternalInput")
    w2_d = nc.dram_tensor("w2", [128, NC10 * 128], F16, kind="ExternalInput")
    w2t_d = nc.dram_tensor("w2t", [128, NC10 * 128], F16,
                           kind="ExternalInput")
    a2g_d = nc.dram_tensor("a2g", [128, NCH * Q], F32, kind="ExternalInput")
    g0_d = nc.dram_tensor("g0", [128, NCH * Q], F16, kind="ExternalInput")
    ae_d = nc.dram_tensor("a_e", [Q, NODES], F32, kind="ExternalInput")
    ssel_d = nc.dram_tensor("s_sel", [Q, CAPS], F16, kind="ExternalInput")
    ident_d = nc.dram_tensor("ident", [128, 128], F16, kind="ExternalInput")
    cpack_d = nc.dram_tensor("cpack", [128, 88], F32, kind="ExternalInput")
    out_d = nc.dram_tensor("out", [BL, CAPS, OUT_DIM], F32,
                           kind="ExternalOutput")
    with tile.TileContext(nc) as tc:
        with ExitStack() as ctx:
            caps_kernel(ctx, tc, out_d.ap(), x_d.ap(),
                        xt_d.ap(), w2_d.ap(), w2t_d.ap(), a2g_d.ap(),
                        g0_d.ap(), ae_d.ap(), ssel_d.ap(), ident_d.ap(),
                        cpack_d.ap())
    nc.compile()
    _CACHE["nc"] = nc
    return nc


def host_prep(W, alpha):
    """Constant input layouts shared by all cores."""
    W = np.asarray(W, dtype=np.float32)
    alpha = np.asarray(alpha, dtype=np.float32)
    w2 = np.ascontiguousarray(
        W.reshape(K, IH, 128, OUT_DIM).transpose(2, 0, 1, 3)
        .reshape(128, NC10 * 128)).astype(np.float16)
    w2t = np.ascontiguousarray(
        W.reshape(K, IH, 128, OUT_DIM).transpose(3, 0, 1, 2)
        .reshape(128, NC10 * 128)).astype(np.float16)
    a2g = np.ascontiguousarray(
        alpha.reshape(NCH, 128, CAPS, K).transpose(1, 0, 3, 2)
        .reshape(128, NCH * Q))
    a_e = np.ascontiguousarray(
        alpha.transpose(2, 1, 0).reshape(Q, NODES))
    s_sel = np.ascontiguousarray(
        np.tile(np.eye(CAPS, dtype=np.float32), (K, 1))).astype(np.float16)
    ident = np.eye(128, dtype=np.float32).astype(np.float16)
    # cpack: ones4 | rc_sel[(b,c),q]=[c==q%CAPS] | fmask[(b,c),b]=[bc//CAPS==b]
    cpack = np.zeros((128, 88), dtype=np.float32)
    cpack[:, 0:4] = 1.0
    cidx = np.arange(BC) % CAPS
    cpack[:BC, 4:84] = (cidx[:, None] == (np.arange(Q) % CAPS)[None, :])
    bidx = np.arange(BC) // CAPS
    cpack[:BC, 84:88] = (bidx[:, None] == np.arange(BL)[None, :])
    g0 = np.ascontiguousarray(a2g * (1.0 / CAPS)).astype(np.float16)
    return {"w2": w2, "w2t": w2t, "a2g": a2g, "g0": g0, "a_e": a_e,
            "s_sel": s_sel, "ident": ident, "cpack": cpack}


def prep_xt(xl):
    """Per-core xT layout [i_local(128), (b, ih, n)]."""
    return np.ascontiguousarray(
        xl.reshape(BL, NODES, IH, 128).transpose(3, 0, 2, 1)
        .reshape(128, BL * IH * NODES))


def make_in_maps(x, W, alpha):
    consts = host_prep(W, alpha)
    in_maps = []
    for c in range(NCORES):
        xl = np.asarray(x, dtype=np.float32)[c * BL:(c + 1) * BL] \
            .astype(np.float16)
        xp = np.ascontiguousarray(
            xl.reshape(BL, NCH, 128, IN_DIM).transpose(2, 0, 1, 3)
            .reshape(128, NG * IN_DIM))
        in_maps.append({"x": xp, "xt": prep_xt(xl), **consts})
    return in_maps


def _enable_ldw_opt():
    # no-op: walrus' LDW optimization rejects the standalone InstLdweights
    # that 16-bit matmuls emit ("InstLdweights is not compatible with LDW
    # optimization"); the fp32r baseline kept it on, but it never deduped
    # anything there either.
    return


def kernel(x, contribution, W, alpha):
    from concourse import bass_utils
    _enable_ldw_opt()

    nc = _build()
    in_maps = make_in_maps(x, W, alpha)
    res = bass_utils.run_bass_kernel_spmd(nc, in_maps,
                                          core_ids=list(range(NCORES)))
    return np.concatenate([res.results[c]["out"] for c in range(NCORES)],
                          axis=0)


# revision 18
# speedup vs baseline: 1.2557x; 1.0040x over previous
"""Trainium2 Bass kernel for the capsule-routing module.

Full-input contract: kernel(**inputs) takes the full [32,...] inputs,
shards batch over 8 NeuronCores (4 per core), runs the Bass kernel via
run_bass_kernel_spmd, and concatenates per-core outputs.

Math (per core, BL=4 local batches):
  Never materializes Wn or u_hat.  With G[n,(k,c)] = c_route[b,c,n] *
  alpha[n,c,k]:
    v[b,c,o]   = sum_{k,i} W[k,i,o] * hT[b][i,(k,c)],
                 hT[b][i,(k,c)] = sum_n x[b,n,i] * G[b][n,(k,c)]
    a[b,c,n]   = sum_k alpha[n,c,k] * e[b][(k,c),n],
                 e[b][(k,c),n] = sum_i wv[b][i,(k,c)] * xT[b][i,n]
                 wv[b][i,(k,c)] = sum_o W[k,i,o] * v[b,c,o]
  All five mm stages run with fp16 inputs / fp32 PSUM accumulation (1
  cyc/row on the PE vs fp32r's 2-4, and halved LDWEIGHTS + DMA bytes).
  With fp16 the routing-flip noise is ~1.6e-2, inside the 2e-2 gate,
  but ONLY if the squash factor sqrt(sn)/(1+sn) is near-exact: a 1e-4
  factor error alone costs ~1.4e-2 (the baseline's exp/ln-table chain
  was the dominant error).  Routing passes therefore refine the factor
  with Newton steps on the DVE (reciprocal_approx_accurate for
  1/(1+sn), one rsqrt-form Newton on the exp/ln seed), and the tiny
  fac-selector matmuls run in full fp32 so nothing re-quantizes it.
  Routing logits accumulate directly in a PSUM bank across passes
  (start at pass 0, stop at the last routing pass); softmax reads the
  running sums in place.  The final-pass output transposes the
  UNSCALED v and applies the (short exp/ln) factor as a per-partition
  scalar afterward, keeping the tail chain off the PE.
"""

import sys

sys.path.insert(0, "/opt/trn_rl_repo")

from contextlib import ExitStack

import numpy as np

import concourse.bacc as bacc
import concourse.mybir as mybir
import concourse.tile as tile

F32 = mybir.dt.float32
F16 = mybir.dt.float16
FR = mybir.dt.float32r
AX = mybir.AxisListType
ALU = mybir.AluOpType
ACTF = mybir.ActivationFunctionType

B, NODES, IN_DIM, OUT_DIM, CAPS, K, NUM_ROUTE = 32, 512, 256, 128, 16, 5, 3
NCORES = 8
BL = B // NCORES          # 4 batches per core
NCH = NODES // 128        # 4 node chunks
IH = IN_DIM // 128        # 2 input-dim chunks
Q = K * CAPS              # 80 = (k,c) packed, q = k*16 + c
NC10 = K * IH             # 10 contraction chunks over (k, ih)
NG = BL * NCH             # 16 softmax groups (b, nch)
BC = BL * CAPS            # 64


def caps_kernel(ctx, tc, out_d, x_d, xt_d, w2_d, w2t_d, a2g_d, g0_d,
                ae_d, ssel_d, ident_d, cpack_d):
    nc = tc.nc

    sb = ctx.enter_context(tc.tile_pool(name="sb", bufs=1))
    work = ctx.enter_context(tc.tile_pool(name="work", bufs=2))
    ps_log = ctx.enter_context(tc.tile_pool(name="ps_log", bufs=1, space="PSUM"))
    ps_h = ctx.enter_context(tc.tile_pool(name="ps_h", bufs=2, space="PSUM"))
    ps_e = ctx.enter_context(tc.tile_pool(name="ps_e", bufs=2, space="PSUM"))
    ps_wv = ctx.enter_context(tc.tile_pool(name="ps_wv", bufs=1, space="PSUM"))
    ps_s = ctx.enter_context(tc.tile_pool(name="ps_s", bufs=1, space="PSUM"))

    # ---------------- persistent SBUF ----------------
    ident = sb.tile([128, 128], F16, tag="ident")

    x_sb = sb.tile([128, NG * IN_DIM], F16, tag="x_sb")          # [p, (b,j,i)]
    xt_sb = sb.tile([128, BL * IH * NODES], F16, tag="xt_sb")    # [i, (b,ih,n)]
    w2 = sb.tile([128, NC10 * 128], F16, tag="w2")               # [i, (c10,o)]
    w2t = sb.tile([128, NC10 * 128], F16, tag="w2t")             # [o, (c10,ki)]
    a2g = sb.tile([128, NCH * Q], F32, tag="a2g")                # [p, (j,k,c)]
    a_e = sb.tile([Q, NODES], F32, tag="a_e")                    # [q, n]
    s_sel = sb.tile([Q, CAPS], F16, tag="s_sel")                 # [q, c]
    g0 = sb.tile([128, NCH * Q], F16, tag="g0")                  # iter-0 G
    cpack = sb.tile([128, 88], F32, tag="cpack")
    ones4 = cpack[:, 0:4]                                        # [128, 4]
    rc_sel = cpack[:BC, 4:84]                                    # [(b,c), q]
    fmask = cpack[:BC, 84:88]                                    # [(b,c), b]
    warm = sb.tile([1, 1], F32, tag="warm")
    # routing logits live in SBUF; per-pass increments are matmul'd into
    # a scratch PSUM bank then added on DVE (PSUM cannot accumulate
    # across closed matmul groups, and reads require closing the group)
    logits = sb.tile([128, NG * CAPS], F32, tag="logits")
    # one shared PSUM bank for all small matmul/transpose outputs
    # (f16 transposes write packed f16: bitcast views over f32 columns)
    small = ps_s.tile([128, 512], F32, tag="small")
    sm_htp = [small[:, 0:40].bitcast(F16), small[:, 40:80].bitcast(F16),
              small[:, 216:256].bitcast(F16), small[:, 256:296].bitcast(F16),
              small[:, 296:336].bitcast(F16), small[:, 336:376].bitcast(F16),
              small[:, 376:416].bitcast(F16), small[:, 416:456].bitcast(F16)]
    sm_vps = small[:, 80:80 + BC]
    sm_snq4 = small[:BC, 144:148]
    sm_snq = small[:BC, 144:145]
    sm_facq = small[:Q, 148:148 + BL]
    sm_outp = small[:BC, 152:152 + 64].bitcast(F16)

    # ---------------- input DMA ----------------
    # Two parallel HWDGE issue queues (Sync + Activation).  Pass-0
    # critical tensors first; one consolidated DMA per tensor (issue
    # cost on the queue is ~600 ns each).  The contribution input is
    # dropped: softmax over caps is invariant to the per-(b,n) constant.
    def xchunk(b):
        return (x_sb[:, b * 1024:(b + 1) * 1024],
                x_d[:, b * 1024:(b + 1) * 1024])

    nc.sync.dma_start(g0[:], g0_d[:, :])
    nc.sync.dma_start(x_sb[:, 0:512], x_d[:, 0:512])
    nc.sync.dma_start(x_sb[:, 512:1024], x_d[:, 512:1024])
    nc.sync.dma_start(*xchunk(1))

    nc.scalar.dma_start(ident[:], ident_d[:, :])
    nc.scalar.dma_start(*xchunk(2))
    nc.scalar.dma_start(w2[:], w2_d[:, :])
    nc.scalar.dma_start(*xchunk(3))
    # warm the ln/exp activation table between DMA issues
    nc.gpsimd.memset(warm[:1, :1], 1.0)
    nc.scalar.activation(warm[:1, :1], warm[:1, :1], ACTF.Ln)
    nc.scalar.dma_start(cpack[:], cpack_d[:, :])
    nc.scalar.dma_start(w2t[:], w2t_d[:, :])
    nc.scalar.dma_start(xt_sb[:], xt_d[:, :])
    nc.scalar.dma_start(a2g[:], a2g_d[:, :])
    nc.scalar.dma_start(a_e[:Q, :], ae_d[:, :])
    nc.scalar.dma_start(s_sel[:Q, :], ssel_d[:, :])
    nc.gpsimd.memset(logits[:], 0.0)

    # ---------------- helpers ----------------
    def spread_copy(idx, dst, src):
        if idx % 2 == 1:
            nc.scalar.copy(dst, src)
        else:
            nc.vector.tensor_copy(dst, src)

    def alloc_softmax():
        return {
            "mx": work.tile([128, NG], F32, tag="mx", name="mx"),
            "sub": work.tile([128, NG * CAPS], F32, tag="sub", name="sub"),
            "exp": work.tile([128, NG * CAPS], F32, tag="exp", name="exp"),
            "sm": work.tile([128, NG], F32, tag="sm", name="sm"),
            "rc": work.tile([128, NG], F32, tag="rc", name="rc"),
            "sn2": work.tile([128, NG * CAPS], F32, tag="sn2", name="sn2"),
            "gt": work.tile([128, NG * Q], F16, tag="gt", name="gt"),
        }

    def emit_softmax_b(b, s):
        # softmax over caps for one batch (reading the PSUM logit sums)
        # + fused G build: gt = (exp*rc) * a2g
        mx, sub, exp, sm, rc, sn2, gt = (s["mx"], s["sub"], s["exp"],
                                         s["sm"], s["rc"], s["sn2"], s["gt"])
        gs = slice(b * NCH, (b + 1) * NCH)
        cs = slice(b * NCH * CAPS, (b + 1) * NCH * CAPS)
        lg3 = logits[:, cs].rearrange("p (g c) -> p g c", g=NCH)
        nc.vector.reduce_max(mx[:, gs], lg3, axis=AX.X)
        nc.vector.tensor_sub(
            sub[:, cs].rearrange("p (g c) -> p g c", g=NCH),
            lg3,
            mx[:, gs].unsqueeze(2).broadcast_to([128, NCH, CAPS]),
        )
        nc.scalar.activation(exp[:, cs], sub[:, cs], ACTF.Exp)
        nc.vector.reduce_sum(
            sm[:, gs],
            exp[:, cs].rearrange("p (g c) -> p g c", g=NCH),
            axis=AX.X)
        nc.vector.reciprocal(rc[:, gs], sm[:, gs])
        nc.vector.tensor_mul(
            sn2[:, cs].rearrange("p (g c) -> p g c", g=NCH),
            exp[:, cs].rearrange("p (g c) -> p g c", g=NCH),
            rc[:, gs].unsqueeze(2).broadcast_to([128, NCH, CAPS]),
        )
        nc.gpsimd.tensor_mul(
            gt[:, b * NCH * Q:(b + 1) * NCH * Q]
            .rearrange("p (g k c) -> p g k c", g=NCH, k=K),
            sn2[:, cs].rearrange("p (g c) -> p g c", g=NCH)
            .unsqueeze(2).broadcast_to([128, NCH, K, CAPS]),
            a2g[:].rearrange("p (g k c) -> p g k c", g=NCH, k=K),
        )

    def fac_seed_chain(sncp, pfx):
        """exp/ln seed f0 = exp(0.5*ln(sn) - ln(1+sn)) on the scalar
        engine (runs concurrently with the DVE reciprocal chain)."""
        lnsn = work.tile([BC, 1], F32, tag=pfx + "lnsn")
        nc.scalar.activation(lnsn[:BC, :], sncp[:BC, :], ACTF.Ln)
        ln1p = work.tile([BC, 1], F32, tag=pfx + "ln1p")
        nc.scalar.activation(ln1p[:BC, :], sncp[:BC, :], ACTF.Ln, bias=1.0)
        arg = work.tile([BC, 1], F32, tag=pfx + "arg")
        nc.vector.scalar_tensor_tensor(arg[:BC, :], lnsn[:BC, :], 0.5,
                                       ln1p[:BC, :],
                                       op0=ALU.mult, op1=ALU.subtract)
        f0 = work.tile([BC, 1], F32, tag=pfx + "f0")
        nc.scalar.activation(f0[:BC, :], arg[:BC, :], ACTF.Exp)
        return f0

    # ---------------- routing ----------------
    cur = None   # softmax tiles for the current pass (None => uniform g0)
    for t in range(NUM_ROUTE + 1):
        fin = (t == NUM_ROUTE)
        if cur is None:
            def g_slice(b, j):
                return g0[:, j * Q:(j + 1) * Q]
        else:
            def g_slice(b, j, gt=cur["gt"]):
                return gt[:, (b * NCH + j) * Q:(b * NCH + j + 1) * Q]

        # --- h[b] = G_b^T @ x_b : psum [q(80) x i(256)] per b; all 16
        # --- h matmuls first, then all 8 PE transposes (keeps the PE
        # --- queue free of copy-stalls) ---
        ht_sb = work.tile([128, BL * IH * Q], F16, tag="ht")
        h_sbs = []
        for b in range(BL):
            hps = ps_h.tile([Q, IN_DIM], F32, tag="hps")
            for j in range(NCH):
                nc.tensor.matmul(
                    hps[:Q, :],
                    g_slice(b, j),
                    x_sb[:, (b * NCH + j) * IN_DIM:
                         (b * NCH + j + 1) * IN_DIM],
                    start=(j == 0),
                    stop=(j == NCH - 1),
                )
            h_sb = work.tile([Q, IN_DIM], F16, tag=f"h{b}")
            if b % 2 == 0:
                nc.scalar.copy(h_sb[:Q, :], hps[:Q, :])
            else:
                nc.vector.tensor_copy(h_sb[:Q, :], hps[:Q, :])
            h_sbs.append(h_sb)
        for b in range(BL):
            for ih in range(IH):
                htp = sm_htp[b * IH + ih]
                nc.tensor.transpose(
                    htp,
                    h_sbs[b][:Q, ih * 128:(ih + 1) * 128],
                    ident[:Q, :Q],
                )
                nc.vector.tensor_copy(
                    ht_sb[:, (b * IH + ih) * Q:(b * IH + ih + 1) * Q], htp)

        # --- V[o, (b,c)] = sum_{k,i} W2[(ki),o] * hT[b][i,(k,c)] ---
        vps = sm_vps
        ht_v = ht_sb[:].rearrange("p (b ih q) -> p b ih q", b=BL, ih=IH)
        for c10 in range(NC10):
            k, ih = divmod(c10, IH)
            nc.tensor.matmul(
                vps.rearrange("p (b c) -> p b c", b=BL),
                w2[:, c10 * 128:(c10 + 1) * 128],
                ht_v[:, :, ih, k * CAPS:(k + 1) * CAPS],
                start=(c10 == 0),
                stop=(c10 == NC10 - 1),
            )
        v_sb = work.tile([128, BC], F16, tag="v_sb")
        nc.scalar.copy(v_sb[:], vps)
        # sn = sum_o v^2 per (b,c), from the fp32 PSUM v (Square is in
        # every act table; also keeps the second PSUM read off the DVE)
        sq = work.tile([128, BC], F32, tag="sq")
        nc.scalar.activation(sq[:], vps, ACTF.Square)

        if fin:
            # transpose the UNSCALED v now (PE), scale by fac afterward
            # as a per-partition scalar
            outp = sm_outp
            nc.tensor.transpose(outp, v_sb[:], ident[:])
            nc.tensor.matmul(sm_snq4, sq[:], ones4, start=True, stop=True)
            sncp = work.tile([BC, 1], F32, tag="sncpf")
            nc.vector.tensor_copy(sncp[:BC, :], sm_snq)
            facx = fac_seed_chain(sncp, "fin_")
            out_sb = work.tile([BC, 128], F32, tag="outsb")
            nc.vector.tensor_scalar(out_sb[:BC, :], outp, facx[:BC, 0:1],
                                    None, op0=ALU.mult)
            nc.sync.dma_start(
                out_d.rearrange("b c o -> (b c) o"),
                out_sb[:BC, :],
            )
            break

        # --- squash factor fac = sqrt(sn)/(1+sn), Newton-refined:
        # ---   r  = 1/(1+sn)   (reciprocal_approx_accurate, ~2 ulp)
        # ---   u  = sn*r^2     (= fac^2)
        # ---   y  = 1/sqrt(u)  (seed 1/f0 from exp/ln chain + 1 Newton)
        # ---   fac = u*y
        # The Newton products run on gpsimd so the DVE queue stays free
        # for the wv spreads; the tiny snq4 matmul is emitted before wv
        # so the chain starts at v-end and hides under wv+e.
        nc.tensor.matmul(sm_snq4, sq[:], ones4, start=True, stop=True)
        sncp = work.tile([BC, 1], F32, tag="sncp")
        nc.vector.tensor_copy(sncp[:BC, :], sm_snq)
        f0 = fac_seed_chain(sncp, "rt_")
        ap1 = work.tile([BC, 1], F32, tag="ap1")
        nc.vector.tensor_scalar(ap1[:BC, :], sncp[:BC, :], 1.0, None,
                                op0=ALU.add)
        rscr = work.tile([BC, 1], F32, tag="rscr")
        rr = work.tile([BC, 1], F32, tag="rr")
        nc.vector.reciprocal_approx_accurate(rr[:BC, :], ap1[:BC, :],
                                             rscr[:BC, :])
        y0 = work.tile([BC, 1], F32, tag="y0")
        nc.vector.reciprocal_approx_fast(y0[:BC, :], f0[:BC, :])

        # --- wv[i, (k,b,c)] = sum_o W[k,i,o] * v[o, (b,c)] (unscaled);
        # --- the DVE Newton ops interleave with the wv spread copies so
        # --- the fac chain and e-feeding both progress ---
        wvp = ps_wv.tile([128, NC10 * BC], F32, tag="wvp")
        for c10 in range(NC10):
            nc.tensor.matmul(
                wvp[:, c10 * BC:(c10 + 1) * BC],
                w2t[:, c10 * 128:(c10 + 1) * 128],
                v_sb[:],
                start=True, stop=True,
            )
        wv_sb = work.tile([128, IH * BL * Q], F16, tag="wv")
        wvp_v = wvp[:].rearrange("p (k ih b c) -> p ih b k c",
                                 k=K, ih=IH, b=BL)

        def wv_spread(b, ih):
            spread_copy(b * IH + ih,
                        wv_sb[:, (ih * BL + b) * Q:(ih * BL + b + 1) * Q]
                        .rearrange("p (k c) -> p k c", k=K),
                        wvp_v[:, ih, b])

        for bb in range(BL):
            wv_spread(bb, 0)
            wv_spread(bb, 1)
        u1 = work.tile([BC, 1], F32, tag="u1")
        nc.vector.tensor_mul(u1[:BC, :], sncp[:BC, :], rr[:BC, :])
        uu = work.tile([BC, 1], F32, tag="uu")
        nc.vector.tensor_mul(uu[:BC, :], u1[:BC, :], rr[:BC, :])
        z1 = work.tile([BC, 1], F32, tag="z1")
        nc.vector.tensor_mul(z1[:BC, :], uu[:BC, :], y0[:BC, :])
        z2 = work.tile([BC, 1], F32, tag="z2")
        nc.vector.tensor_mul(z2[:BC, :], z1[:BC, :], y0[:BC, :])
        wn = work.tile([BC, 1], F32, tag="wn")
        nc.vector.tensor_scalar(wn[:BC, :], z2[:BC, :], -0.5, 1.5,
                                op0=ALU.mult, op1=ALU.add)
        y1 = work.tile([BC, 1], F32, tag="y1")
        nc.vector.tensor_mul(y1[:BC, :], y0[:BC, :], wn[:BC, :])
        facx = work.tile([BC, 1], F32, tag="facx")
        nc.vector.tensor_mul(facx[:BC, :], uu[:BC, :], y1[:BC, :])
        # facq[q, b] = fac[b, c(q)] via a constant fp32 selector matmul;
        # the matmul itself is emitted inside the e pipeline (after
        # e(b1)) so the fac chain never blocks the PE
        rhsm = work.tile([BC, BL], F32, tag="rhsm")
        nc.vector.tensor_mul(rhsm[:BC, :],
                             facx[:BC, 0:1].broadcast_to([BC, BL]),
                             fmask)
        facq = work.tile([Q, BL], F32, tag="facqs")

        # --- e[b] = wv_b^T @ xT_b : [q(80) x n(512)];
        # --- tmp = e * fac[b,c(q)] * alpha; logit mms accumulate into
        # --- the persistent aps PSUM regions; then immediately emit the
        # --- NEXT pass's softmax for this b ---
        aps = ps_log.tile([128, NG * CAPS], F32, tag="aps")
        nxt = alloc_softmax()
        epss = {}

        def emit_e(b):
            eps = ps_e.tile([Q, NODES], F32, tag="eps")
            for ih in range(IH):
                nc.tensor.matmul(
                    eps[:Q, :],
                    wv_sb[:, (ih * BL + b) * Q:(ih * BL + b + 1) * Q],
                    xt_sb[:, (b * IH + ih) * NODES:
                          (b * IH + ih + 1) * NODES],
                    start=(ih == 0),
                    stop=(ih == IH - 1),
                )
            epss[b] = eps

        def emit_tmp_log(b):
            eps = epss[b]
            tmp = work.tile([Q, NODES], F16, tag="tmp")
            # halved so the first logit matmuls start after half an STT
            for hs, js in ((slice(0, 256), (0, 1)),
                           (slice(256, 512), (2, 3))):
                nc.vector.scalar_tensor_tensor(tmp[:Q, hs], eps[:Q, hs],
                                               facq[:Q, b:b + 1],
                                               a_e[:Q, hs],
                                               op0=ALU.mult, op1=ALU.mult)
                for j in js:
                    g = b * NCH + j
                    nc.tensor.matmul(
                        aps[:, g * CAPS:(g + 1) * CAPS],
                        tmp[:Q, j * 128:(j + 1) * 128],
                        s_sel[:Q, :],
                        start=True, stop=True,
                    )
            bs = slice(b * NCH * CAPS, (b + 1) * NCH * CAPS)
            nc.vector.tensor_add(logits[:, bs], logits[:, bs], aps[:, bs])
            emit_softmax_b(b, nxt)

        emit_e(0)
        emit_e(1)
        nc.tensor.matmul(sm_facq, rc_sel, rhsm[:BC, :],
                         start=True, stop=True)
        nc.vector.tensor_copy(facq[:Q, :], sm_facq)
        emit_tmp_log(0)
        emit_e(2)
        emit_tmp_log(1)
        emit_e(3)
        emit_tmp_log(2)
        emit_tmp_log(3)
        cur = nxt


_CACHE = {}


def _force_combined_exp_ln_table(arch):
    """Make natural_log_exp_and_others the only act set offering Exp/Ln so
    the table-load pass never alternates tables between softmax (Exp) and
    the squash-factor seed (Ln)."""
    from concourse.hw_specs import get_activation_tables
    try:
        tabs = get_activation_tables(arch)
    except Exception:
        return
    for name, s in tabs.items():
        if name != "natural_log_exp_and_others":
            s.discard(ACTF.Exp)
            s.discard(ACTF.Ln)


def _build():
    if "nc" in _CACHE:
        return _CACHE["nc"]
    nc = bacc.Bacc("TRN2", target_bir_lowering=False, debug=False,
                   num_devices=NCORES)
    _force_combined_exp_ln_table(nc.m.arch)
    x_d = nc.dram_tensor("x", [128, NG * IN_DIM], F16, kind="ExternalInput")
    xt_d = nc.dram_tensor("xt", [128, BL * IH * NODES], F16,
                          kind="ExternalInput")
    w2_d = nc.dram_tensor("w2", [128, NC10 * 128], F16, kind="ExternalInput")
    w2t_d = nc.dram_tensor("w2t", [128, NC10 * 128], F16,
                           kind="ExternalInput")
    a2g_d = nc.dram_tensor("a2g", [128, NCH * Q], F32, kind="ExternalInput")
    g0_d = nc.dram_tensor("g0", [128, NCH * Q], F16, kind="ExternalInput")
    ae_d = nc.dram_tensor("a_e", [Q, NODES], F32, kind="ExternalInput")
    ssel_d = nc.dram_tensor("s_sel", [Q, CAPS], F16, kind="ExternalInput")
    ident_d = nc.dram_tensor("ident", [128, 128], F16, kind="ExternalInput")
    cpack_d = nc.dram_tensor("cpack", [128, 88], F32, kind="ExternalInput")
    out_d = nc.dram_tensor("out", [BL, CAPS, OUT_DIM], F32,
                           kind="ExternalOutput")
    with tile.TileContext(nc) as tc:
        with ExitStack() as ctx:
            caps_kernel(ctx, tc, out_d.ap(), x_d.ap(),
                        xt_d.ap(), w2_d.ap(), w2t_d.ap(), a2g_d.ap(),
                        g0_d.ap(), ae_d.ap(), ssel_d.ap(), ident_d.ap(),
                        cpack_d.ap())
    nc.compile()
    _CACHE["nc"] = nc
    return nc


def host_prep(W, alpha):
    """Constant input layouts shared by all cores."""
    W = np.asarray(W, dtype=np.float32)
    alpha = np.asarray(alpha, dtype=np.float32)
    w2 = np.ascontiguousarray(
        W.reshape(K, IH, 128, OUT_DIM).transpose(2, 0, 1, 3)
        .reshape(128, NC10 * 128)).astype(np.float16)
    w2t = np.ascontiguousarray(
        W.reshape(K, IH, 128, OUT_DIM).transpose(3, 0, 1, 2)
        .reshape(128, NC10 * 128)).astype(np.float16)
    a2g = np.ascontiguousarray(
        alpha.reshape(NCH, 128, CAPS, K).transpose(1, 0, 3, 2)
        .reshape(128, NCH * Q))
    a_e = np.ascontiguousarray(
        alpha.transpose(2, 1, 0).reshape(Q, NODES))
    s_sel = np.ascontiguousarray(
        np.tile(np.eye(CAPS, dtype=np.float32), (K, 1))).astype(np.float16)
    ident = np.eye(128, dtype=np.float32).astype(np.float16)
    # cpack: ones4 | rc_sel[(b,c),q]=[c==q%CAPS] | fmask[(b,c),b]=[bc//CAPS==b]
    cpack = np.zeros((128, 88), dtype=np.float32)
    cpack[:, 0:4] = 1.0
    cidx = np.arange(BC) % CAPS
    cpack[:BC, 4:84] = (cidx[:, None] == (np.arange(Q) % CAPS)[None, :])
    bidx = np.arange(BC) // CAPS
    cpack[:BC, 84:88] = (bidx[:, None] == np.arange(BL)[None, :])
    g0 = np.ascontiguousarray(a2g * (1.0 / CAPS)).astype(np.float16)
    return {"w2": w2, "w2t": w2t, "a2g": a2g, "g0": g0, "a_e": a_e,
            "s_sel": s_sel, "ident": ident, "cpack": cpack}


def prep_xt(xl):
    """Per-core xT layout [i_local(128), (b, ih, n)]."""
    return np.ascontiguousarray(
        xl.reshape(BL, NODES, IH, 128).transpose(3, 0, 2, 1)
        .reshape(128, BL * IH * NODES))


def make_in_maps(x, W, alpha):
    consts = host_prep(W, alpha)
    in_maps = []
    for c in range(NCORES):
        xl = np.asarray(x, dtype=np.float32)[c * BL:(c + 1) * BL] \
            .astype(np.float16)
        xp = np.ascontiguousarray(
            xl.reshape(BL, NCH, 128, IN_DIM).transpose(2, 0, 1, 3)
            .reshape(128, NG * IN_DIM))
        in_maps.append({"x": xp, "xt": prep_xt(xl), **consts})
    return in_maps


def _enable_ldw_opt():
    # no-op: walrus' LDW optimization rejects the standalone InstLdweights
    # that 16-bit matmuls emit ("InstLdweights is not compatible with LDW
    # optimization"); the fp32r baseline kept it on, but it never deduped
    # anything there either.
    return


def kernel(x, contribution, W, alpha):
    from concourse import bass_utils
    _enable_ldw_opt()

    nc = _build()
    in_maps = make_in_maps(x, W, alpha)
    res = bass_utils.run_bass_kernel_spmd(nc, in_maps,
                                          core_ids=list(range(NCORES)))
    return np.concatenate([res.results[c]["out"] for c in range(NCORES)],
                          axis=0)
